# revision 6
# baseline (speedup 1.0000x reference)
"""Trainium2 Bass kernel for nn_ContrastiveConceptualAlignmentLoss.

loss = mean over {i2t, t2i} of KL(softmax(jaccard_sim/T) || softmax(logits))
computed as in the reference (F.kl_div(log_probs, target, 'batchmean')).

Strategy (8-way SPMD, row-sharded):
  Each core holds full concepts [512,512], computes a [64,512] slab of the
  pairwise-Jaccard similarity and of both KL terms, reduces to one scalar
  partial; the host sums the 8 partials (the all-reduce) and scales.

Key device-side identities (c = relu(concepts) is 0/1 valued):
  inter = c @ c.T                 (TensorE, bf16 inputs exact, f32 accum)
  union = s_i + s_j - inter       (s = row sums of c, fused into the relu)
  1-sim = (union-inter)/union  -> E = exp(-(1-sim)/T)  [softmax shift=1/T,
           exactly the per-row max the reference subtracts, since sim_ii=1]
  KL row sums need only Z=sum(E), Aprod=sum(E*(1-sim)), B=sum(E*logits),
  Zl=sum(exp(logits)) -- all fused into ACT accum_out / DVE stt accum_out.

The [64,512] slab is packed as [128,256] (column halves stacked on
partitions) so every vector op uses all 128 lanes.
"""

import json

import numpy as np

TEMP = 0.07
B = 512
C = 512
M = 8
SLAB = B // M  # 64
H = 2          # column halves per slab row block
HW = 256       # half width


# ---------------------------------------------------------------------------
# Workaround for walrus "Too many sync wait commands": split multi-wait
# instructions into standalone single-wait EventSemaphore instructions.
# ---------------------------------------------------------------------------

def _split_multiwaits_json(bir_bytes: bytes) -> bytes:
    bir = json.loads(bir_bytes)
    changed = False
    for fn in bir.get("functions", []):
        for blk in fn.get("blocks", []):
            new_insts = []
            ctr = 0
            for ins in blk.get("instructions", []):
                si = ins.get("sync_info")
                if si:
                    waits = si.get("on_wait") or []
                    if len(waits) > 1:
                        for w in waits[:-1]:
                            ctr += 1
                            new_insts.append({
                                "name": f"{ins['name']}_xw{ctr}",
                                "engine": ins["engine"],
                                "opcode": "EventSemaphore",
                                "ins": [],
                                "outs": [],
                                "debug": ins.get("debug", 0),
                                "sync_info": {"on_wait": [w], "on_update": []},
                            })
                        si["on_wait"] = [waits[-1]]
                        changed = True
                new_insts.append(ins)
            blk["instructions"] = new_insts
    return json.dumps(bir).encode() if changed else bir_bytes


_patched = False


def _install_birpatch():
    global _patched
    if _patched:
        return
    _patched = True
    from concourse import bass2jax, bass_utils

    orig = bass_utils.compile_bir_kernel

    def patched(bir_json, tmpdir, neff_name="file.neff"):
        return orig(_split_multiwaits_json(bir_json), tmpdir, neff_name)

    bass_utils.compile_bir_kernel = patched
    bass2jax.compile_bir_kernel = patched


# ---------------------------------------------------------------------------
# Kernel program (identical on all 8 cores; per-core data differs)
# ---------------------------------------------------------------------------

def build_nc():
    import concourse.bass as bass
    import concourse.tile as tile
    from concourse import mybir
    from concourse.masks import make_identity

    f32 = mybir.dt.float32
    bf16 = mybir.dt.bfloat16
    Alu = mybir.AluOpType
    Act = mybir.ActivationFunctionType

    nc = bass.Bass()
    concepts = nc.dram_tensor("concepts", [B, C], f32, kind="ExternalInput")
    srow_dram = nc.dram_tensor("srow_scratch", [4, 128], f32)
    cslab = nc.dram_tensor("cslab", [SLAB, C], f32, kind="ExternalInput")
    l1p = nc.dram_tensor("l1p", [128, HW], f32, kind="ExternalInput")
    l2p = nc.dram_tensor("l2p", [128, HW], f32, kind="ExternalInput")
    out = nc.dram_tensor("out", [1, 1], f32, kind="ExternalOutput")

    with tile.TileContext(nc) as tc:
        with (
            tc.tile_pool(name="consts", bufs=1) as consts,
            tc.tile_pool(name="sb", bufs=1) as sb,
            tc.tile_pool(name="tp_ps", bufs=2, space="PSUM") as tp_ps,
            tc.tile_pool(name="mm_ps", bufs=1, space="PSUM") as mm_ps,
        ):
            id_bf = consts.tile([128, 128], bf16, tag="id_bf")
            make_identity(nc, id_bf)
            id_f32 = consts.tile([128, 128], f32, tag="id_f32")
            make_identity(nc, id_f32)

            # ---- input DMAs (split row-tiles across queues for overlap) ----
            conc_t = []
            for t in range(4):
                ct = sb.tile([128, C], f32, tag=f"conc{t}")
                nc.sync.dma_start(out=ct, in_=concepts[t * 128:(t + 1) * 128, :])
                conc_t.append(ct)
            cslab_t = sb.tile([SLAB, C], f32, tag="cslab")
            nc.sync.dma_start(out=cslab_t, in_=cslab[:, :])
            l1t = sb.tile([128, HW], f32, tag="l1t")
            nc.sync.dma_start(out=l1t, in_=l1p[:, :])
            l2t = sb.tile([128, HW], f32, tag="l2t")
            nc.sync.dma_start(out=l2t, in_=l2p[:, :])

            # ---- relu -> c (bf16) with fused row-sums s ----
            s_cols = sb.tile([128, 4], f32, tag="s_cols")
            c_bf = []
            for t in range(4):
                cb = sb.tile([128, C], bf16, tag=f"c_bf{t}")
                nc.scalar.activation(out=cb, in_=conc_t[t], func=Act.Relu,
                                     accum_out=s_cols[:, t:t + 1])
                c_bf.append(cb)
            cslab_bf = sb.tile([SLAB, C], bf16, tag="cslab_bf")
            s_slab = sb.tile([SLAB, 1], f32, tag="s_slab")
            nc.scalar.activation(out=cslab_bf, in_=cslab_t, func=Act.Relu,
                                 accum_out=s_slab)

            # ---- transpose c -> cT (16 PE transposes via bf16 identity) ----
            cT = []
            for kk in range(4):
                tpp = tp_ps.tile([128, C], bf16, tag="tp")
                for t in range(4):
                    nc.tensor.transpose(
                        tpp[:, t * 128:(t + 1) * 128],
                        c_bf[t][:, kk * 128:(kk + 1) * 128],
                        id_bf,
                    )
                cTk = sb.tile([128, C], bf16, tag=f"cT{kk}")
                # PSUM -> SBUF must go through a compute engine; balance ACT/DVE
                if kk % 2 == 0:
                    nc.scalar.copy(out=cTk, in_=tpp)
                else:
                    nc.vector.tensor_copy(out=cTk, in_=tpp)
                cT.append(cTk)

            # ---- transpose slab -> lhsT [128k, 64i] x 4 ktiles ----
            slabT = tp_ps.tile([128, 4 * SLAB], bf16, tag="slabT")
            for kk in range(4):
                nc.tensor.transpose(
                    slabT[:, kk * SLAB:(kk + 1) * SLAB],
                    cslab_bf[:, kk * 128:(kk + 1) * 128],
                    id_bf[0:SLAB, 0:SLAB],
                )
            lhsT = sb.tile([128, 4 * SLAB], bf16, tag="lhsT")
            nc.vector.tensor_copy(out=lhsT, in_=slabT)

            # ---- s as a row: transpose s_cols [128,4] -> [4,128] ----
            srow_ps = mm_ps.tile([4, 128], f32, tag="srow")
            nc.tensor.transpose(srow_ps, s_cols, id_f32)
            srow_sb = sb.tile([4, 128], f32, tag="srow_sb")
            nc.scalar.copy(out=srow_sb, in_=srow_ps)
            nc.sync.dma_start(out=srow_dram[:, :], in_=srow_sb)

            # ---- sj broadcast tile [128, 256]: partition p reads s[h(p)*256+f]
            # partition-broadcast (step-0) APs are only legal on DRAM sources,
            # hence the round-trip through srow_dram.
            sj = sb.tile([128, HW], f32, tag="sj")
            for h in range(H):
                for tt in range(2):
                    row = 2 * h + tt
                    src = srow_dram[row:row + 1, :]
                    bsrc = bass.AP(tensor=src.tensor, offset=src.offset,
                                   ap=[[0, SLAB]] + [list(p) for p in src.ap[1:]])
                    nc.gpsimd.dma_start(
                        out=sj[SLAB * h:SLAB * (h + 1), tt * 128:(tt + 1) * 128],
                        in_=bsrc,
                    )

            # ---- s_slab duplicated on both partition halves ----
            s_slab2 = sb.tile([128, 1], f32, tag="s_slab2")
            nc.gpsimd.dma_start(out=s_slab2[0:SLAB, :], in_=s_slab)
            nc.gpsimd.dma_start(out=s_slab2[SLAB:128, :], in_=s_slab)

            # ---- inter = c_slab @ c.T, packed [128, 256] ----
            inter_ps = mm_ps.tile([128, HW], f32, tag="inter")
            for h in range(H):
                for kk in range(4):
                    nc.tensor.matmul(
                        inter_ps[SLAB * h:SLAB * (h + 1), :],
                        lhsT=lhsT[:, kk * SLAB:(kk + 1) * SLAB],
                        rhs=cT[kk][:, HW * h:HW * (h + 1)],
                        start=(kk == 0),
                        stop=(kk == 3),
                    )

            # ---- DVE chain ----
            # u = inter - s_i - s_j = -union
            u = sb.tile([128, HW], f32, tag="u")
            nc.vector.scalar_tensor_tensor(out=u, in0=inter_ps, scalar=s_slab2,
                                           in1=sj, op0=Alu.subtract,
                                           op1=Alu.subtract)
            # negd = inter + u = -(union - inter)
            negd = sb.tile([128, HW], f32, tag="negd")
            nc.vector.tensor_add(out=negd, in0=inter_ps, in1=u)
            rr = sb.tile([128, HW], f32, tag="rr")
            nc.vector.reciprocal(out=rr, in_=u)  # = -1/union
            # prod = 1 - sim  (>= 0)
            prod = sb.tile([128, HW], f32, tag="prod")
            nc.vector.tensor_mul(out=prod, in0=negd, in1=rr)

            # E = exp(-prod/T), Z fused
            hact = sb.tile([128, 3], f32, tag="hact")
            e_sb = sb.tile([128, HW], f32, tag="e_sb")
            nc.scalar.activation(out=e_sb, in_=prod, func=Act.Exp,
                                 scale=-1.0 / TEMP, accum_out=hact[:, 0:1])

            hdve = sb.tile([128, 3], f32, tag="hdve")
            scr = sb.tile([128, HW], f32, tag="scr")
            nc.vector.scalar_tensor_tensor(out=scr, in0=e_sb, scalar=1.0,
                                           in1=prod, op0=Alu.mult, op1=Alu.mult,
                                           accum_out=hdve[:, 0:1])
            nc.vector.scalar_tensor_tensor(out=scr, in0=e_sb, scalar=1.0,
                                           in1=l1t, op0=Alu.mult, op1=Alu.mult,
                                           accum_out=hdve[:, 1:2])
            nc.vector.scalar_tensor_tensor(out=scr, in0=e_sb, scalar=1.0,
                                           in1=l2t, op0=Alu.mult, op1=Alu.mult,
                                           accum_out=hdve[:, 2:3])

            scr2 = sb.tile([128, HW], f32, tag="scr2")
            nc.scalar.activation(out=scr2, in_=l1t, func=Act.Exp,
                                 accum_out=hact[:, 1:2])
            nc.scalar.activation(out=scr2, in_=l2t, func=Act.Exp,
                                 accum_out=hact[:, 2:3])

            # ---- combine column halves: X = X_h[0:64] + X_h[64:128] ----
            hact_hi = sb.tile([SLAB, 3], f32, tag="hact_hi")
            nc.gpsimd.dma_start(out=hact_hi, in_=hact[SLAB:128, :])
            hdve_hi = sb.tile([SLAB, 3], f32, tag="hdve_hi")
            nc.gpsimd.dma_start(out=hdve_hi, in_=hdve[SLAB:128, :])
            stats_a = sb.tile([SLAB, 3], f32, tag="stats_a")
            nc.vector.tensor_add(out=stats_a, in0=hact[0:SLAB, :], in1=hact_hi)
            stats_d = sb.tile([SLAB, 3], f32, tag="stats_d")
            nc.vector.tensor_add(out=stats_d, in0=hdve[0:SLAB, :], in1=hdve_hi)

            # ---- per-row tail  ([64,1] ops) ----
            # stats_a: [Z, Zl1, Zl2];  stats_d: [Aprod, B1, B2]
            rz = sb.tile([SLAB, 1], f32, tag="rz")
            nc.vector.reciprocal(out=rz, in_=stats_a[:, 0:1])
            lns = sb.tile([SLAB, 3], f32, tag="lns")
            nc.scalar.activation(out=lns, in_=stats_a, func=Act.Ln)
            n1 = sb.tile([SLAB, 1], f32, tag="n1")
            nc.vector.scalar_tensor_tensor(out=n1, in0=stats_d[:, 0:1],
                                           scalar=2.0 / TEMP,
                                           in1=stats_d[:, 1:2],
                                           op0=Alu.mult, op1=Alu.add)
            n12 = sb.tile([SLAB, 1], f32, tag="n12")
            nc.vector.tensor_add(out=n12, in0=n1, in1=stats_d[:, 2:3])
            n3 = sb.tile([SLAB, 1], f32, tag="n3")
            nc.vector.tensor_mul(out=n3, in0=n12, in1=rz)
            m1 = sb.tile([SLAB, 1], f32, tag="m1")
            nc.vector.scalar_tensor_tensor(out=m1, in0=lns[:, 0:1], scalar=2.0,
                                           in1=lns[:, 1:2],
                                           op0=Alu.mult, op1=Alu.subtract)
            m2 = sb.tile([SLAB, 1], f32, tag="m2")
            nc.vector.tensor_tensor(out=m2, in0=m1, in1=lns[:, 2:3],
                                    op=Alu.subtract)
            rowv = sb.tile([SLAB, 1], f32, tag="rowv")
            nc.vector.tensor_add(out=rowv, in0=n3, in1=m2)

            # ---- partial = sum_i rowv * (-1/(2B)) via PE ones-matmul ----
            ones = nc.const_aps.tensor(1.0, (SLAB, 1), f32)
            res_ps = mm_ps.tile([1, 1], f32, tag="res")
            nc.tensor.matmul(res_ps, lhsT=rowv, rhs=ones)
            res_sb = sb.tile([1, 1], f32, tag="res_sb")
            nc.scalar.mul(out=res_sb, in_=res_ps, mul=-1.0 / (2.0 * B))
            nc.sync.dma_start(out=out[:, :], in_=res_sb)

    return nc


def make_in_maps(logits_per_image, logits_per_text, medical_concepts):
    conc = np.ascontiguousarray(np.asarray(medical_concepts, dtype=np.float32))
    l1 = np.asarray(logits_per_image, dtype=np.float32)
    l2 = np.asarray(logits_per_text, dtype=np.float32)
    in_maps = []
    for k in range(M):
        rb = k * SLAB
        in_maps.append({
            "concepts": conc,
            "cslab": np.ascontiguousarray(conc[rb:rb + SLAB, :]),
            "l1p": np.ascontiguousarray(
                np.concatenate([l1[rb:rb + SLAB, :HW], l1[rb:rb + SLAB, HW:]], axis=0)),
            "l2p": np.ascontiguousarray(
                np.concatenate([l2[rb:rb + SLAB, :HW], l2[rb:rb + SLAB, HW:]], axis=0)),
        })
    return in_maps


_nc_cache = None


def _get_nc():
    global _nc_cache
    if _nc_cache is None:
        _nc_cache = build_nc()
    return _nc_cache


def kernel(logits_per_image, logits_per_text, medical_concepts):
    _install_birpatch()
    from concourse.bass_utils import run_bass_kernel_spmd

    nc = _get_nc()
    in_maps = make_in_maps(logits_per_image, logits_per_text, medical_concepts)
    res = run_bass_kernel_spmd(nc, in_maps, list(range(M)))
    total = np.float32(0.0)
    for k in range(M):
        total += np.float32(res.results[k]["out"][0, 0])
    return np.array(total, dtype=np.float32)
